# revision 4
# baseline (speedup 1.0000x reference)
"""Trainium2 Bass kernel for the mixed low-rank-expert DCN-v2 block (nn_DCN_51539607711).

Reference math (L=3 layers, E=4 experts, D=512, R=64, B=16384):
  x_{l+1} = sum_e x0 * (tanh(tanh(x_l V_e) C_e) U_e^T + b_l) * gate_e + x_l
The gate is softmax over a size-1 axis == exactly 1.0, so G never affects the
output. With gate == 1 the recurrence telescopes:
  x_{l+1} = x0 * (1 + sum_{i<=l} (A_i(x_i) + E*b_i)),
  A_i(x) = sum_e U_e tanh(C_e^T tanh(V_e^T x))
so the residual stream is carried as a single PSUM accumulator s = sum_i A_i
(fp32, accumulated by the PE across all experts AND layers), and each layer's
activation update is ONE fused DVE op per chunk:
  x_{l+1} = (s + c_l[d]) * x0,   c_l = 1 + E*cumsum(b)_l   (per-partition scalar)

Distribution: pure data-parallel over B across 8 cores (2048 rows/core),
weights replicated. Activations live feature-major (xT: [D, B]); every matmul
contracts on the partition dim with zero on-device transposes. Experts are
packed in pairs to fill all 128 partitions/output rows:
  v-step :  vT[pair]  = Vpair^T  @ xlT    (lhsT = Vpair [D,128], K=D in 4 chunks)
  cv-step:  cvT[pair] = blockdiag(C_e0,C_e1)^T @ vT[pair]   (K=128)
  ucv    :  s[mchunk] += Upair^T-packed @ cvT[pair]          (K=128, accum)
All matmul operands bf16 (fp32 PSUM accumulation); x0 is kept only in bf16
(rel-err budget is 2e-2; bf16 x0 costs ~3e-3).

DMA plan: x is host-packed block-major ([P, NBLK, KC, NB] bf16, 4KB lines) and
loaded with one DMA per block on the Sync queue; weights are host-packed
layer-major ([vw|cw|uw] per layer) and loaded with one DMA per layer on the
Scalar queue so the transfer-arrival order matches first-use order; outputs
store block-major from the GpSimd queue. Three queues run in parallel, so
block 0's x and layer 0's weights land ~4us in instead of waiting behind the
whole weight blob.
"""

import numpy as np
import ml_dtypes

import concourse.bacc as bacc
import concourse.tile as tile
from concourse import mybir
from concourse.bass_utils import run_bass_kernel_spmd

L, E, D, R, B = 3, 4, 512, 64, 16384
NCORES = 8
BC = B // NCORES          # batch columns per core (2048)
NB = 512                  # block of batch columns (one PSUM bank at fp32)
NBLK = BC // NB           # blocks per core
P = 128                   # partitions
KC = D // P               # contraction chunks over D (4)
NPAIR = E // 2            # expert pairs (2)

F32 = mybir.dt.float32
BF16 = mybir.dt.bfloat16
bf16 = ml_dtypes.bfloat16

# per-layer weight region: [vw (NPAIR*KC*P) | cw (NPAIR*P) | uw (NPAIR*D)]
VW_L = NPAIR * KC * P     # 1024
CW_L = NPAIR * P          # 256
UW_L = NPAIR * D          # 1024
WL_COLS = VW_L + CW_L + UW_L   # 2304
WB_COLS = L * WL_COLS          # 6912

_CACHE = {}


def _build_nc(bc=BC):
    """Build the per-core Bass/Tile kernel. Identical NEFF on all cores."""
    nblk = bc // NB
    nc = bacc.Bacc("TRN2", target_bir_lowering=False, debug=False,
                   num_devices=NCORES)

    xq_d = nc.dram_tensor("xq", [P, nblk, KC, NB], BF16, kind="ExternalInput")
    wb_d = nc.dram_tensor("wb", [P, WB_COLS], BF16, kind="ExternalInput")
    cb_d = nc.dram_tensor("cb", [P, L, KC], F32, kind="ExternalInput")
    oq_d = nc.dram_tensor("oq", [P, nblk, KC, NB], F32, kind="ExternalOutput")

    Tanh = mybir.ActivationFunctionType.Tanh
    ADD = mybir.AluOpType.add
    MULT = mybir.AluOpType.mult

    with tile.TileContext(nc) as tc:
        with (
            tc.tile_pool(name="wpool", bufs=1) as wpool,
            tc.tile_pool(name="xpool", bufs=1) as xpool,
            tc.tile_pool(name="xl_pool", bufs=3) as xl_pool,
            tc.tile_pool(name="vt_pool", bufs=3) as vt_pool,
            tc.tile_pool(name="cvt_pool", bufs=3) as cvt_pool,
            tc.tile_pool(name="out_pool", bufs=2) as out_pool,
            tc.tile_pool(name="psum_s", bufs=4, space="PSUM") as psum_s,
            tc.tile_pool(name="psum_t", bufs=2, space="PSUM") as psum_t,
        ):
            # ---- persistent inputs. x blocks on the Sync DMA queue,
            # weight layers on the Scalar queue, cb on the GpSimd queue:
            # three queues transfer in parallel, so block 0 + layer 0
            # arrive first and compute starts ~4us in.
            xq_s = xpool.tile([P, nblk, KC, NB], BF16)
            wb_s = wpool.tile([P, WB_COLS], BF16)
            cb_s = wpool.tile([P, L, KC], F32)
            nc.sync.dma_start(xq_s[:, 0], xq_d[:, 0])
            nc.scalar.dma_start(wb_s[:, 0:WL_COLS], wb_d[:, 0:WL_COLS])
            nc.gpsimd.dma_start(cb_s[:], cb_d[:])
            for b in range(1, nblk):
                nc.sync.dma_start(xq_s[:, b], xq_d[:, b])
            for l in range(1, L):
                nc.scalar.dma_start(wb_s[:, l * WL_COLS:(l + 1) * WL_COLS],
                                    wb_d[:, l * WL_COLS:(l + 1) * WL_COLS])

            def wv(l):
                base = l * WL_COLS
                return wb_s[:, base:base + VW_L].rearrange(
                    "p (q k m) -> p q k m", q=NPAIR, k=KC)

            def wc(l):
                base = l * WL_COLS + VW_L
                return wb_s[:, base:base + CW_L].rearrange(
                    "p (q m) -> p q m", q=NPAIR)

            def wu(l):
                base = l * WL_COLS + VW_L + CW_L
                return wb_s[:, base:base + UW_L].rearrange(
                    "p (q m) -> p q m", q=NPAIR)

            for b in range(nblk):
                s_tiles = [psum_s.tile([P, NB], F32, name=f"s_{b}_{m}", tag="s")
                           for m in range(KC)]
                xl_cur = [xq_s[:, b, k] for k in range(KC)]

                for l in range(L):
                    # v = tanh(Vpair^T @ xl). k-major pair-interleaved
                    # emission: consumes xl chunks in the order the previous
                    # layer's STTs produce them, so the PE never waits at a
                    # layer boundary.
                    vp = psum_t.tile([P, NPAIR, NB], F32,
                                     name=f"vp_{b}_{l}", tag="pt")
                    for k in range(KC):
                        for p in range(NPAIR):
                            nc.tensor.matmul(vp[:, p, :], wv(l)[:, p, k],
                                             xl_cur[k],
                                             start=(k == 0), stop=(k == KC - 1))
                    vt = vt_pool.tile([P, NPAIR, NB], BF16,
                                      name=f"vt_{b}_{l}", tag="vt")
                    for p in range(NPAIR):
                        nc.scalar.activation(vt[:, p, :], vp[:, p, :], Tanh)
                    # cv = tanh(blockdiag(C)^T @ v)
                    cp = psum_t.tile([P, NPAIR, NB], F32,
                                     name=f"cp_{b}_{l}", tag="pt")
                    for p in range(NPAIR):
                        nc.tensor.matmul(cp[:, p, :], wc(l)[:, p], vt[:, p, :],
                                         start=True, stop=True)
                    cvt = cvt_pool.tile([P, NPAIR, NB], BF16,
                                        name=f"cvt_{b}_{l}", tag="cvt")
                    for p in range(NPAIR):
                        nc.scalar.activation(cvt[:, p, :], cp[:, p, :], Tanh)
                    # s[m] += Upacked^T @ cv  (accumulates across pairs AND
                    # layers). start=True clears has_written for the whole
                    # bank, so only the bank's very first matmul starts;
                    # stop closes the sim's group so the DVE may read s;
                    # later layers bypass the sim group check (HW accumulates
                    # via per-element has_written bits regardless).
                    # Emission (m0p0)(m1p0)(m0p1)(m1p1)(m2p0)(m2p1)(m3p0)
                    # (m3p1): p0-only head hides cvt-p1's tanh latency, while
                    # m0p1/m1p1 still come early so the STTs start early.
                    uorder = [(0, 0), (1, 0), (0, 1), (1, 1),
                              (2, 0), (2, 1), (3, 0), (3, 1)]
                    for m, p in uorder:
                        nc.tensor.matmul(
                            s_tiles[m],
                            wu(l)[:, p, m * P:(m + 1) * P],
                            cvt[:, p, :],
                            start=(l == 0 and p == 0),
                            stop=(l == 0 and p == 1),
                            skip_group_check=(l > 0),
                        )
                    # x_{l+1} = (s + c_l) * x0 on the DVE (ACT does only
                    # tanhs); final layer writes fp32 out and stores in two
                    # halves so the first DMA starts early.
                    if l < L - 1:
                        xln = xl_pool.tile([P, KC, NB], BF16,
                                           name=f"xl_{b}_{l}", tag="xl")
                        for m in range(KC):
                            nc.vector.scalar_tensor_tensor(
                                xln[:, m, :], s_tiles[m], cb_s[:, l, m:m + 1],
                                xq_s[:, b, m], ADD, MULT)
                        xl_cur = [xln[:, k, :] for k in range(KC)]
                    else:
                        ot = out_pool.tile([P, KC, NB], F32,
                                           name=f"ot_{b}", tag="ot")
                        for m in range(KC):
                            nc.vector.scalar_tensor_tensor(
                                ot[:, m, :], s_tiles[m], cb_s[:, l, m:m + 1],
                                xq_s[:, b, m], ADD, MULT)
                            if m == 1:
                                nc.gpsimd.dma_start(oq_d[:, b, 0:2],
                                                    ot[:, 0:2, :])
                        nc.gpsimd.dma_start(oq_d[:, b, 2:4], ot[:, 2:4, :])

    nc.compile()
    return nc


def _prep_weights(U, V, C, bias):
    """Host-side packing into the exact SBUF layouts (see module docstring)."""
    wb = np.empty([P, WB_COLS], dtype=bf16)
    for l in range(L):
        base = l * WL_COLS
        VwH = np.empty([P, NPAIR, KC, P], dtype=bf16)
        CwH = np.zeros([P, NPAIR, P], dtype=bf16)
        UwH = np.empty([P, NPAIR, D], dtype=bf16)
        for p in range(NPAIR):
            vpair = np.concatenate([V[l, 2 * p], V[l, 2 * p + 1]], axis=1)  # [D,128]
            VwH[:, p, :, :] = vpair.reshape(KC, P, P).transpose(1, 0, 2)
            upair = np.concatenate([U[l, 2 * p].T, U[l, 2 * p + 1].T], axis=0)  # [128,D]
            UwH[:, p, :] = upair
            CwH[:R, p, :R] = C[l, 2 * p]
            CwH[R:, p, R:] = C[l, 2 * p + 1]
        wb[:, base:base + VW_L] = VwH.reshape(P, VW_L)
        wb[:, base + VW_L:base + VW_L + CW_L] = CwH.reshape(P, CW_L)
        wb[:, base + VW_L + CW_L:base + WL_COLS] = UwH.reshape(P, UW_L)
    cb = 1.0 + E * np.cumsum(bias.astype(np.float32), axis=0)       # [L, D]
    cbH = np.ascontiguousarray(
        cb.reshape(L, KC, P).transpose(2, 0, 1)).astype(np.float32)  # [P, L, KC]
    return np.ascontiguousarray(wb), cbH


def _make_in_maps(x, U, V, C, G, bias):
    wbH, cbH = _prep_weights(np.asarray(U, np.float32),
                             np.asarray(V, np.float32),
                             np.asarray(C, np.float32),
                             np.asarray(bias, np.float32))
    # xq[core][p, b, k, j] = x[core*BC + b*NB + j, k*128 + p]  (bf16)
    xT = np.asarray(x, np.float32).T.astype(bf16)       # [D, B]
    xq = xT.reshape(KC, P, NCORES, NBLK, NB).transpose(2, 1, 3, 0, 4)
    in_maps = []
    for c in range(NCORES):
        in_maps.append({
            "xq": np.ascontiguousarray(xq[c]),
            "wb": wbH, "cb": cbH,
        })
    return in_maps


def _run(inputs, trace=False, **kw):
    key = "nc"
    if key not in _CACHE:
        _CACHE[key] = _build_nc()
    nc = _CACHE[key]
    in_maps = _make_in_maps(**inputs)
    res = run_bass_kernel_spmd(nc, in_maps, core_ids=list(range(NCORES)),
                               trace=trace, **kw)
    # oq[core][p, b, m, j] -> out[core*BC + b*NB + j, m*128 + p]
    out = np.empty((B, D), np.float32)
    for c in range(NCORES):
        oq = res.results[c]["oq"]                        # [P, NBLK, KC, NB]
        out[c * BC:(c + 1) * BC, :] = (
            oq.transpose(1, 3, 2, 0).reshape(BC, D))
    return out, res


def kernel(**inputs) -> np.ndarray:
    out, _ = _run(inputs, trace=False)
    return out


# revision 8
# speedup vs baseline: 1.3734x; 1.3734x over previous
"""Trainium2 Bass kernel for the mixed low-rank-expert DCN-v2 block (nn_DCN_51539607711).

Reference math (L=3 layers, E=4 experts, D=512, R=64, B=16384):
  x_{l+1} = sum_e x0 * (tanh(tanh(x_l V_e) C_e) U_e^T + b_l) * gate_e + x_l
The gate is softmax over a size-1 axis == exactly 1.0, so G never affects the
output. With gate == 1 the recurrence telescopes:
  x_{l+1} = x0 * (1 + sum_{i<=l} (A_i(x_i) + E*b_i)),
  A_i(x) = sum_e U_e tanh(C_e^T tanh(V_e^T x))
so the residual stream is carried as a single PSUM accumulator s = sum_i A_i
(fp32, accumulated by the PE across all experts AND layers), and each layer's
activation update is ONE fused DVE op per chunk:
  x_{l+1} = (s + c_l[d]) * x0,   c_l = 1 + E*cumsum(b)_l   (per-partition scalar)

Distribution: pure data-parallel over B across 8 cores (2048 rows/core),
weights replicated. Activations live feature-major (xT: [D, B]); every matmul
contracts on the partition dim with zero on-device transposes. Experts are
packed in pairs to fill all 128 partitions/output rows:
  v-step :  vT[pair]  = Vpair^T  @ xlT    (lhsT = Vpair [D,128], K=D in 4 chunks)
  cv-step:  cvT[pair] = blockdiag(C_e0,C_e1)^T @ vT[pair]   (K=128)
  ucv    :  s[mchunk] += Upair^T-packed @ cvT[pair]          (K=128, accum)
All matmul operands bf16 (fp32 PSUM accumulation); x0 is kept only in bf16
(rel-err budget is 2e-2; bf16 x0 costs ~3e-3).

DMA plan: x is host-packed block-major ([P, NBLK, KC, NB] bf16, 4KB lines) and
loaded with one DMA per block on the Sync queue; weights are host-packed
layer-major ([vw|cw|uw] per layer) and loaded with one DMA per layer on the
Scalar queue so the transfer-arrival order matches first-use order; outputs
store block-major from the GpSimd queue. Three queues run in parallel, so
block 0's x and layer 0's weights land ~4us in instead of waiting behind the
whole weight blob.
"""

import numpy as np
import ml_dtypes

import concourse.bacc as bacc
import concourse.tile as tile
from concourse import mybir
from concourse.bass_utils import run_bass_kernel_spmd

L, E, D, R, B = 3, 4, 512, 64, 16384
NCORES = 8
BC = B // NCORES          # batch columns per core (2048)
NB = 512                  # block of batch columns (one PSUM bank at fp32)
NBLK = BC // NB           # blocks per core
P = 128                   # partitions
KC = D // P               # contraction chunks over D (4)
NPAIR = E // 2            # expert pairs (2)

F32 = mybir.dt.float32
BF16 = mybir.dt.bfloat16
bf16 = ml_dtypes.bfloat16

# per-layer weight region: [vw (NPAIR*KC*P) | cw (NPAIR*P) | uw (NPAIR*D)]
VW_L = NPAIR * KC * P     # 1024
CW_L = NPAIR * P          # 256
UW_L = NPAIR * D          # 1024
WL_COLS = VW_L + CW_L + UW_L   # 2304
WB_COLS = L * WL_COLS          # 6912

_CACHE = {}


def _build_nc(bc=BC):
    """Build the per-core Bass/Tile kernel. Identical NEFF on all cores."""
    nblk = bc // NB
    nc = bacc.Bacc("TRN2", target_bir_lowering=False, debug=False,
                   num_devices=NCORES)

    xq_d = nc.dram_tensor("xq", [P, nblk, KC, NB], BF16, kind="ExternalInput")
    wb_d = nc.dram_tensor("wb", [P, WB_COLS], BF16, kind="ExternalInput")
    cb_d = nc.dram_tensor("cb", [P, L, KC], F32, kind="ExternalInput")
    oq_d = nc.dram_tensor("oq", [P, nblk, KC, NB], F32, kind="ExternalOutput")

    Tanh = mybir.ActivationFunctionType.Tanh
    ADD = mybir.AluOpType.add
    MULT = mybir.AluOpType.mult

    with tile.TileContext(nc) as tc:
        with (
            tc.tile_pool(name="wpool", bufs=1) as wpool,
            tc.tile_pool(name="xpool", bufs=1) as xpool,
            tc.tile_pool(name="xl_pool", bufs=10) as xl_pool,
            tc.tile_pool(name="vt_pool", bufs=4) as vt_pool,
            tc.tile_pool(name="cvt_pool", bufs=4) as cvt_pool,
            tc.tile_pool(name="out_pool", bufs=2) as out_pool,
            tc.tile_pool(name="psum_s", bufs=4, space="PSUM") as psum_s,
            tc.tile_pool(name="psum_t", bufs=4, space="PSUM") as psum_t,
        ):
            # ---- persistent inputs. x blocks on the Sync DMA queue,
            # weight layers on the Scalar queue, cb on the GpSimd queue:
            # three queues transfer in parallel. The very first pieces the
            # PE needs (x block 0 chunk 0, vw layer 0) get their own small
            # DMAs so the first matmul isn't gated on a big transfer.
            xq_s = xpool.tile([P, nblk, KC, NB], BF16)
            wb_s = wpool.tile([P, WB_COLS], BF16)
            cb_s = wpool.tile([P, L, KC], F32)
            for k in range(KC):
                nc.sync.dma_start(xq_s[:, 0, k], xq_d[:, 0, k])
            nc.scalar.dma_start(wb_s[:, 0:VW_L], wb_d[:, 0:VW_L])
            nc.scalar.dma_start(wb_s[:, VW_L:WL_COLS], wb_d[:, VW_L:WL_COLS])
            nc.gpsimd.dma_start(cb_s[:], cb_d[:])
            for b in range(1, nblk):
                nc.sync.dma_start(xq_s[:, b], xq_d[:, b])
            for l in range(1, L):
                nc.scalar.dma_start(wb_s[:, l * WL_COLS:(l + 1) * WL_COLS],
                                    wb_d[:, l * WL_COLS:(l + 1) * WL_COLS])

            def wv(l):
                base = l * WL_COLS
                return wb_s[:, base:base + VW_L].rearrange(
                    "p (q k m) -> p q k m", q=NPAIR, k=KC)

            def wc(l):
                base = l * WL_COLS + VW_L
                return wb_s[:, base:base + CW_L].rearrange(
                    "p (q m) -> p q m", q=NPAIR)

            def wu(l):
                base = l * WL_COLS + VW_L + CW_L
                return wb_s[:, base:base + UW_L].rearrange(
                    "p (q m) -> p q m", q=NPAIR)

            for b in range(nblk):
                s_tiles = [psum_s.tile([P, NB], F32, name=f"s_{b}_{m}", tag="s")
                           for m in range(KC)]
                xl_cur = [xq_s[:, b, k] for k in range(KC)]

                for l in range(L):
                    # v = tanh(Vpair^T @ xl). k-major pair-interleaved
                    # emission: consumes xl chunks in the order the previous
                    # layer's STTs produce them, so the PE never waits at a
                    # layer boundary.
                    vps = [psum_t.tile([P, NB], F32, name=f"vp_{b}_{l}_{p}",
                                       tag="pt") for p in range(NPAIR)]
                    for k in range(KC):
                        for p in range(NPAIR):
                            nc.tensor.matmul(vps[p][:], wv(l)[:, p, k],
                                             xl_cur[k],
                                             start=(k == 0), stop=(k == KC - 1))
                    vts = []
                    for p in range(NPAIR):
                        vt = vt_pool.tile([P, NB], BF16,
                                          name=f"vt_{b}_{l}_{p}", tag="vt")
                        nc.scalar.activation(vt[:], vps[p][:], Tanh)
                        vts.append(vt)
                    # cv = tanh(blockdiag(C)^T @ v)
                    cvts = []
                    for p in range(NPAIR):
                        cps = psum_t.tile([P, NB], F32, name=f"cp_{b}_{l}_{p}",
                                          tag="pt")
                        nc.tensor.matmul(cps[:], wc(l)[:, p], vts[p][:],
                                         start=True, stop=True)
                        cvt = cvt_pool.tile([P, NB], BF16,
                                            name=f"cvt_{b}_{l}_{p}", tag="cvt")
                        nc.scalar.activation(cvt[:], cps[:], Tanh)
                        cvts.append(cvt)
                    # s[m] += Upacked^T @ cv  (accumulates across pairs AND
                    # layers). start=True clears has_written for the whole
                    # bank, so only the bank's very first matmul starts;
                    # stop closes the sim's group so the DVE may read s;
                    # later layers bypass the sim group check (HW accumulates
                    # via per-element has_written bits regardless).
                    # Emission (m0p0)(m1p0)(m0p1)(m1p1)(m2p0)(m2p1)(m3p0)
                    # (m3p1): p0-only head hides cvt-p1's tanh latency, while
                    # m0p1/m1p1 still come early so the STTs start early.
                    uorder = [(0, 0), (1, 0), (0, 1), (1, 1),
                              (2, 0), (2, 1), (3, 0), (3, 1)]
                    for m, p in uorder:
                        nc.tensor.matmul(
                            s_tiles[m],
                            wu(l)[:, p, m * P:(m + 1) * P],
                            cvts[p][:],
                            start=(l == 0 and p == 0),
                            stop=(l == 0 and p == 1),
                            skip_group_check=(l > 0),
                        )
                    # x_{l+1} = (s + c_l) * x0 on the DVE (ACT does only
                    # tanhs); final layer writes fp32 out and stores in two
                    # halves so the first DMA starts early.
                    if l < L - 1:
                        nxt = []
                        for m in range(KC):
                            xln = xl_pool.tile([P, NB], BF16,
                                               name=f"xl_{b}_{l}_{m}", tag="xl")
                            nc.vector.scalar_tensor_tensor(
                                xln[:], s_tiles[m], cb_s[:, l, m:m + 1],
                                xq_s[:, b, m], ADD, MULT)
                            nxt.append(xln)
                        xl_cur = [t[:] for t in nxt]
                    else:
                        ot = out_pool.tile([P, KC, NB], F32,
                                           name=f"ot_{b}", tag="ot")
                        for m in range(KC):
                            nc.vector.scalar_tensor_tensor(
                                ot[:, m, :], s_tiles[m], cb_s[:, l, m:m + 1],
                                xq_s[:, b, m], ADD, MULT)
                            if m == 1:
                                nc.gpsimd.dma_start(oq_d[:, b, 0:2],
                                                    ot[:, 0:2, :])
                        nc.gpsimd.dma_start(oq_d[:, b, 2:4], ot[:, 2:4, :])

    nc.compile()
    return nc


def _prep_weights(U, V, C, bias):
    """Host-side packing into the exact SBUF layouts (see module docstring)."""
    wb = np.empty([P, WB_COLS], dtype=bf16)
    for l in range(L):
        base = l * WL_COLS
        VwH = np.empty([P, NPAIR, KC, P], dtype=bf16)
        CwH = np.zeros([P, NPAIR, P], dtype=bf16)
        UwH = np.empty([P, NPAIR, D], dtype=bf16)
        for p in range(NPAIR):
            vpair = np.concatenate([V[l, 2 * p], V[l, 2 * p + 1]], axis=1)  # [D,128]
            VwH[:, p, :, :] = vpair.reshape(KC, P, P).transpose(1, 0, 2)
            upair = np.concatenate([U[l, 2 * p].T, U[l, 2 * p + 1].T], axis=0)  # [128,D]
            UwH[:, p, :] = upair
            CwH[:R, p, :R] = C[l, 2 * p]
            CwH[R:, p, R:] = C[l, 2 * p + 1]
        wb[:, base:base + VW_L] = VwH.reshape(P, VW_L)
        wb[:, base + VW_L:base + VW_L + CW_L] = CwH.reshape(P, CW_L)
        wb[:, base + VW_L + CW_L:base + WL_COLS] = UwH.reshape(P, UW_L)
    cb = 1.0 + E * np.cumsum(bias.astype(np.float32), axis=0)       # [L, D]
    cbH = np.ascontiguousarray(
        cb.reshape(L, KC, P).transpose(2, 0, 1)).astype(np.float32)  # [P, L, KC]
    return np.ascontiguousarray(wb), cbH


def _make_in_maps(x, U, V, C, G, bias):
    wbH, cbH = _prep_weights(np.asarray(U, np.float32),
                             np.asarray(V, np.float32),
                             np.asarray(C, np.float32),
                             np.asarray(bias, np.float32))
    # xq[core][p, b, k, j] = x[core*BC + b*NB + j, k*128 + p]  (bf16)
    xT = np.asarray(x, np.float32).T.astype(bf16)       # [D, B]
    xq = xT.reshape(KC, P, NCORES, NBLK, NB).transpose(2, 1, 3, 0, 4)
    in_maps = []
    for c in range(NCORES):
        in_maps.append({
            "xq": np.ascontiguousarray(xq[c]),
            "wb": wbH, "cb": cbH,
        })
    return in_maps


def _run(inputs, trace=False, **kw):
    key = "nc"
    if key not in _CACHE:
        _CACHE[key] = _build_nc()
    nc = _CACHE[key]
    in_maps = _make_in_maps(**inputs)
    res = run_bass_kernel_spmd(nc, in_maps, core_ids=list(range(NCORES)),
                               trace=trace, **kw)
    # oq[core][p, b, m, j] -> out[core*BC + b*NB + j, m*128 + p]
    out = np.empty((B, D), np.float32)
    for c in range(NCORES):
        oq = res.results[c]["oq"]                        # [P, NBLK, KC, NB]
        out[c * BC:(c + 1) * BC, :] = (
            oq.transpose(1, 3, 2, 0).reshape(BC, D))
    return out, res


def kernel(**inputs) -> np.ndarray:
    out, _ = _run(inputs, trace=False)
    return out
